# revision 17
# baseline (speedup 1.0000x reference)
"""ConvLSTM decoder (2 ConvLSTM layers + top conv) on 8 Trainium2 cores.

Sharding: data-parallel over batch — B=8, one batch element per core,
weights replicated. The T=10 recurrence runs fully on-core.

Layout: images are stored in SBUF as a zero-padded flat row-major strip:
each 64-pixel row padded to 66 cols (1 zero col each side), 64 rows
contiguous, plus 68-col zero margins at both ends. A 3x3 'SAME' conv then
becomes 9 shifted matmuls accumulated in PSUM: for tap (dy,dx) the rhs is
the flat strip shifted by dy*66+dx.
"""

import numpy as np

B, T, C, H, W = 8, 10, 64, 64, 64
CH = 128
NSTEP = T - 1          # 9 recurrent steps
WP = W + 2             # padded row width
FLAT = H * WP          # 4224
MARG = 68              # >= 67 = max |tap offset|
BUFC = MARG + FLAT + MARG
BASE = MARG
HW = H * W             # 4096

# row chunks (r0, r1): 8x7 rows + 2x4 rows; max matmul N = 7*66 = 462 <= 512
CHUNKS = [(i * 7, i * 7 + 7) for i in range(8)] + [(56, 60), (60, 64)]

TAPS = [(dy, dx) for dy in (-1, 0, 1) for dx in (-1, 0, 1)]

MM_DT = "f32r"         # "f32" | "f32r" | "bf16"
LOOP_N = 0             # >0: wrap body in a hardware repeat loop (timing only)

_CACHE = {}


def _np_dt(mybir):
    if MM_DT == "bf16":
        return mybir.dt.bfloat16
    if MM_DT == "f32r":
        return mybir.dt.float32r
    return mybir.dt.float32


def _prep_w(w):
    # [O, I, 3, 3] -> [I, 9*O]; slice for (tap ti, 128-chunk g): ti*O + g*128
    O, I = w.shape[0], w.shape[1]
    return np.ascontiguousarray(
        w.transpose(1, 2, 3, 0).reshape(I, 9 * O).astype(np.float32))


def _build():
    import concourse.bass as bass
    import concourse.tile as tile
    from concourse import bacc, mybir

    f32 = mybir.dt.float32
    cdt = _np_dt(mybir)          # matmul-input dtype in SBUF
    AF = mybir.ActivationFunctionType

    nc = bacc.Bacc("TRN2", target_bir_lowering=False, debug=False,
                   num_devices=8)

    ddt = cdt if MM_DT == "f32r" else f32   # DRAM dtype for matmul operands
    xs_d = nc.dram_tensor("xs", [NSTEP, C, HW], ddt, kind="ExternalInput")
    h0_d = nc.dram_tensor("h0i", [CH, HW], ddt, kind="ExternalInput")
    c0_d = nc.dram_tensor("c0i", [CH, HW], f32, kind="ExternalInput")
    h1_d = nc.dram_tensor("h1i", [CH, HW], ddt, kind="ExternalInput")
    c1_d = nc.dram_tensor("c1i", [CH, HW], f32, kind="ExternalInput")
    w0_d = nc.dram_tensor("w0", [C, 9 * 4 * CH], ddt, kind="ExternalInput")
    u0_d = nc.dram_tensor("u0", [CH, 9 * 4 * CH], ddt, kind="ExternalInput")
    w1_d = nc.dram_tensor("w1", [CH, 9 * 4 * CH], ddt, kind="ExternalInput")
    u1_d = nc.dram_tensor("u1", [CH, 9 * 4 * CH], ddt, kind="ExternalInput")
    wt_d = nc.dram_tensor("wt", [CH, 9 * C], ddt, kind="ExternalInput")
    zz_d = nc.dram_tensor("zz", [CH, BUFC], ddt, kind="ExternalInput")
    b0_d = nc.dram_tensor("b0", [CH, 4], f32, kind="ExternalInput")
    b1_d = nc.dram_tensor("b1", [CH, 4], f32, kind="ExternalInput")
    bt_d = nc.dram_tensor("bt", [C, 1], f32, kind="ExternalInput")
    out_d = nc.dram_tensor("out", [T, C, HW], f32, kind="ExternalOutput")

    def interior(ap_2d, s0, nrow):
        # rows of 64 interior cols at stride 66 starting at flat offset s0
        return ap_2d[:, s0:s0 + nrow * WP].rearrange(
            "p (r c) -> p r c", c=WP)[:, :, 1:1 + W]

    with tile.TileContext(nc) as tc:
        with (
            tc.tile_pool(name="pers", bufs=1) as pers,
            tc.tile_pool(name="ps", bufs=8, space="PSUM") as psp,
            tc.tile_pool(name="gt", bufs=2) as gtp,
            tc.tile_pool(name="osb", bufs=1) as osbp,
        ):
            # --- persistent SBUF residents ---
            w0_t = pers.tile([C, 9 * 4 * CH], cdt, tag="w0")
            u0_t = pers.tile([CH, 9 * 4 * CH], cdt, tag="u0")
            w1_t = pers.tile([CH, 9 * 4 * CH], cdt, tag="w1")
            u1_t = pers.tile([CH, 9 * 4 * CH], cdt, tag="u1")
            wt_t = pers.tile([CH, 9 * C], cdt, tag="wt")
            b0_t = pers.tile([CH, 4], f32, tag="b0")
            b1_t = pers.tile([CH, 4], f32, tag="b1")
            bt_t = pers.tile([C, 1], f32, tag="bt")
            xbuf = pers.tile([C, BUFC], cdt, tag="xb")
            h0p = [pers.tile([CH, BUFC], cdt, tag=f"h0p{i}", name=f"h0p{i}")
                   for i in range(2)]
            h1p = [pers.tile([CH, BUFC], cdt, tag=f"h1p{i}", name=f"h1p{i}")
                   for i in range(2)]
            c0_t = pers.tile([CH, HW], f32, tag="c0")
            c1_t = pers.tile([CH, HW], f32, tag="c1")

            dma = (nc.gpsimd if MM_DT == "bf16" else nc.sync)

            for t_, d_ in ((w0_t, w0_d), (u0_t, u0_d), (w1_t, w1_d),
                           (u1_t, u1_d), (wt_t, wt_d)):
                dma.dma_start(t_[:], d_.ap())
            for t_, d_ in ((b0_t, b0_d), (b1_t, b1_d), (bt_t, bt_d)):
                nc.sync.dma_start(t_[:], d_.ap())

            def init_states():
                for buf in (xbuf, h0p[0], h0p[1], h1p[0], h1p[1]):
                    if MM_DT == "f32r":
                        np_ = buf.shape[0]
                        nc.sync.dma_start(buf[:], zz_d.ap()[:np_])
                    else:
                        nc.vector.memset(buf[:], 0.0)
                dma.dma_start(interior(h0p[0], BASE, H), h0_d.ap())
                dma.dma_start(interior(h1p[0], BASE, H), h1_d.ap())
                nc.sync.dma_start(c0_t[:], c0_d.ap())
                nc.sync.dma_start(c1_t[:], c1_d.ap())

            def mm_cast(ap):
                return ap

            def conv_gates(xin, kx, wi_t, hin, wh_t, b_t, c_t, hout):
                """One ConvLSTM cell: gates = conv(xin)+conv(hin)+b; update
                c_t in place; write new h into hout's interior."""
                for (r0, r1) in CHUNKS:
                    nr = r1 - r0
                    cw = nr * WP
                    nin = nr * W
                    gts = []
                    for g in range(4):
                        ps = psp.tile([CH, cw], f32, tag="ps")
                        mms = []
                        for ti in range(9):
                            dy, dx = TAPS[ti]
                            s = BASE + r0 * WP + dy * WP + dx
                            o = ti * 4 * CH + g * CH
                            mms.append((wi_t[:kx, o:o + CH], xin[:kx, s:s + cw]))
                        for ti in range(9):
                            dy, dx = TAPS[ti]
                            s = BASE + r0 * WP + dy * WP + dx
                            o = ti * 4 * CH + g * CH
                            mms.append((wh_t[:, o:o + CH], hin[:, s:s + cw]))
                        if hin is not xbuf and xin is not xbuf:
                            # layer 1: h2h operand ready earlier; put it first
                            mms = mms[9:] + mms[:9]
                        for k, (lhs, rhs) in enumerate(mms):
                            nc.tensor.matmul(ps[:], mm_cast(lhs), mm_cast(rhs),
                                             start=(k == 0), stop=(k == 17))
                        gt = gtp.tile([CH, nin], f32, tag=f"g{g}")
                        func = AF.Tanh if g == 2 else AF.Sigmoid
                        nc.scalar.activation(
                            gt[:].rearrange("p (r c) -> p r c", c=W),
                            ps[:].rearrange("p (r c) -> p r c", c=WP)[:, :, 1:1 + W],
                            func, bias=b_t[:, g:g + 1])
                        gts.append(gt)
                    gi, gf, gg, go = gts
                    csl = c_t[:, r0 * W:r1 * W]
                    nc.vector.tensor_mul(gg[:], gi[:], gg[:])   # i*g
                    nc.vector.tensor_mul(csl, gf[:], csl)       # f*c
                    nc.vector.tensor_add(csl, csl, gg[:])       # c = f*c + i*g
                    nc.scalar.activation(gf[:], csl, AF.Tanh)   # tanh(c) -> gf
                    nc.vector.tensor_mul(
                        interior(hout, BASE + r0 * WP, nr),
                        go[:].rearrange("p (r c) -> p r c", c=W),
                        gf[:].rearrange("p (r c) -> p r c", c=W))

            def conv_top(hin, tout):
                for (r0, r1) in CHUNKS:
                    nr = r1 - r0
                    cw = nr * WP
                    nin = nr * W
                    ps = psp.tile([C, cw], f32, tag="ps")
                    for ti in range(9):
                        dy, dx = TAPS[ti]
                        s = BASE + r0 * WP + dy * WP + dx
                        nc.tensor.matmul(ps[:], mm_cast(wt_t[:, ti * C:(ti + 1) * C]),
                                         mm_cast(hin[:, s:s + cw]),
                                         start=(ti == 0), stop=(ti == 8))
                    ot = osbp.tile([C, nin], f32, tag="ot")
                    nc.scalar.activation(
                        ot[:].rearrange("p (r c) -> p r c", c=W),
                        ps[:].rearrange("p (r c) -> p r c", c=WP)[:, :, 1:1 + W],
                        AF.Identity, bias=bt_t[:, 0:1])
                    nc.sync.dma_start(tout[:, r0 * W:r1 * W], ot[:])

            def body():
                init_states()
                conv_top(h1p[0], out_d.ap()[0])
                for t in range(NSTEP):
                    dma.dma_start(interior(xbuf, BASE, H), xs_d.ap()[t])
                    conv_gates(xbuf, C, w0_t, h0p[t % 2], u0_t, b0_t, c0_t,
                               h0p[(t + 1) % 2])
                    conv_gates(h0p[(t + 1) % 2], CH, w1_t, h1p[t % 2], u1_t,
                               b1_t, c1_t, h1p[(t + 1) % 2])
                    conv_top(h1p[(t + 1) % 2], out_d.ap()[t + 1])

            if LOOP_N > 0:
                with tc.For_i(0, LOOP_N, 1):
                    body()
            else:
                body()

    nc.compile()
    return nc


def _get_nc():
    if "nc" not in _CACHE:
        _CACHE["nc"] = _build()
    return _CACHE["nc"]


def kernel(target, h0, c0, h1, c1,
           wi0, bi0, wh0, bh0,
           wi1, bi1, wh1, bh1,
           wtop, btop):
    from concourse.bass_utils import run_bass_kernel_spmd

    nc = _get_nc()

    target = np.asarray(target, np.float32)
    shared = {
        "w0": _prep_w(np.asarray(wi0, np.float32)),
        "u0": _prep_w(np.asarray(wh0, np.float32)),
        "w1": _prep_w(np.asarray(wi1, np.float32)),
        "u1": _prep_w(np.asarray(wh1, np.float32)),
        "wt": _prep_w(np.asarray(wtop, np.float32)),
        "b0": np.ascontiguousarray(
            (np.asarray(bi0) + np.asarray(bh0)).astype(np.float32)
            .reshape(4, CH).T),
        "b1": np.ascontiguousarray(
            (np.asarray(bi1) + np.asarray(bh1)).astype(np.float32)
            .reshape(4, CH).T),
        "bt": np.asarray(btop, np.float32).reshape(C, 1),
        "zz": np.zeros((CH, BUFC), np.float32),
    }
    in_maps = []
    for b in range(B):
        m = dict(shared)
        m["xs"] = np.ascontiguousarray(
            target[b, :NSTEP].reshape(NSTEP, C, HW))
        m["h0i"] = np.ascontiguousarray(np.asarray(h0, np.float32)[b].reshape(CH, HW))
        m["c0i"] = np.ascontiguousarray(np.asarray(c0, np.float32)[b].reshape(CH, HW))
        m["h1i"] = np.ascontiguousarray(np.asarray(h1, np.float32)[b].reshape(CH, HW))
        m["c1i"] = np.ascontiguousarray(np.asarray(c1, np.float32)[b].reshape(CH, HW))
        in_maps.append(m)

    res = run_bass_kernel_spmd(nc, in_maps, core_ids=list(range(B)))
    out = np.stack([res.results[b]["out"].reshape(T, C, H, W)
                    for b in range(B)])
    return out


# revision 19
# speedup vs baseline: 1.0769x; 1.0769x over previous
"""ConvLSTM decoder (2 ConvLSTM layers + top conv) on 8 Trainium2 cores.

Sharding: data-parallel over batch — B=8, one batch element per core,
weights replicated. The T=10 recurrence runs fully on-core.

Layout: images are stored in SBUF as a zero-padded flat row-major strip:
each 64-pixel row padded to 66 cols (1 zero col each side), 64 rows
contiguous, plus 68-col zero margins at both ends. A 3x3 'SAME' conv then
becomes 9 shifted matmuls accumulated in PSUM: for tap (dy,dx) the rhs is
the flat strip shifted by dy*66+dx.
"""

import numpy as np

B, T, C, H, W = 8, 10, 64, 64, 64
CH = 128
NSTEP = T - 1          # 9 recurrent steps
WP = W + 2             # padded row width
FLAT = H * WP          # 4224
MARG = 68              # >= 67 = max |tap offset|
BUFC = MARG + FLAT + MARG
BASE = MARG
HW = H * W             # 4096

# row chunks (r0, r1): 8x7 rows + 2x4 rows; max matmul N = 7*66 = 462 <= 512
CHUNKS = [(i * 7, i * 7 + 7) for i in range(8)] + [(56, 60), (60, 64)]

TAPS = [(dy, dx) for dy in (-1, 0, 1) for dx in (-1, 0, 1)]

MM_DT = "f32r"         # "f32" | "f32r" | "bf16"
LOOP_N = 0             # >0: wrap body in a hardware repeat loop (timing only)

_CACHE = {}


def _np_dt(mybir):
    if MM_DT == "bf16":
        return mybir.dt.bfloat16
    if MM_DT == "f32r":
        return mybir.dt.float32r
    return mybir.dt.float32


def _prep_w(w):
    # [O, I, 3, 3] -> [I, 9*O]; slice for (tap ti, 128-chunk g): ti*O + g*128
    O, I = w.shape[0], w.shape[1]
    return np.ascontiguousarray(
        w.transpose(1, 2, 3, 0).reshape(I, 9 * O).astype(np.float32))


def _build():
    import concourse.bass as bass
    import concourse.tile as tile
    from concourse import bacc, mybir

    f32 = mybir.dt.float32
    cdt = _np_dt(mybir)          # matmul-input dtype in SBUF
    AF = mybir.ActivationFunctionType

    nc = bacc.Bacc("TRN2", target_bir_lowering=False, debug=False,
                   num_devices=8)

    ddt = cdt if MM_DT == "f32r" else f32   # DRAM dtype for matmul operands
    xs_d = nc.dram_tensor("xs", [NSTEP, C, HW], ddt, kind="ExternalInput")
    h0_d = nc.dram_tensor("h0i", [CH, HW], ddt, kind="ExternalInput")
    c0_d = nc.dram_tensor("c0i", [CH, HW], f32, kind="ExternalInput")
    h1_d = nc.dram_tensor("h1i", [CH, HW], ddt, kind="ExternalInput")
    c1_d = nc.dram_tensor("c1i", [CH, HW], f32, kind="ExternalInput")
    w0_d = nc.dram_tensor("w0", [C, 9 * 4 * CH], ddt, kind="ExternalInput")
    u0_d = nc.dram_tensor("u0", [CH, 9 * 4 * CH], ddt, kind="ExternalInput")
    w1_d = nc.dram_tensor("w1", [CH, 9 * 4 * CH], ddt, kind="ExternalInput")
    u1_d = nc.dram_tensor("u1", [CH, 9 * 4 * CH], ddt, kind="ExternalInput")
    wt_d = nc.dram_tensor("wt", [CH, 9 * C], ddt, kind="ExternalInput")
    zz_d = nc.dram_tensor("zz", [CH, BUFC], ddt, kind="ExternalInput")
    b0_d = nc.dram_tensor("b0", [CH, 4], f32, kind="ExternalInput")
    b1_d = nc.dram_tensor("b1", [CH, 4], f32, kind="ExternalInput")
    bt_d = nc.dram_tensor("bt", [C, 1], f32, kind="ExternalInput")
    out_d = nc.dram_tensor("out", [T, C, HW], f32, kind="ExternalOutput")

    def interior(ap_2d, s0, nrow):
        # rows of 64 interior cols at stride 66 starting at flat offset s0
        return ap_2d[:, s0:s0 + nrow * WP].rearrange(
            "p (r c) -> p r c", c=WP)[:, :, 1:1 + W]

    with tile.TileContext(nc) as tc:
        with (
            tc.tile_pool(name="pers", bufs=1) as pers,
            tc.tile_pool(name="ps", bufs=8, space="PSUM") as psp,
            tc.tile_pool(name="gt", bufs=2) as gtp,
            tc.tile_pool(name="osb", bufs=1) as osbp,
        ):
            # --- persistent SBUF residents ---
            w0_t = pers.tile([C, 9 * 4 * CH], cdt, tag="w0")
            u0_t = pers.tile([CH, 9 * 4 * CH], cdt, tag="u0")
            w1_t = pers.tile([CH, 9 * 4 * CH], cdt, tag="w1")
            u1_t = pers.tile([CH, 9 * 4 * CH], cdt, tag="u1")
            wt_t = pers.tile([CH, 9 * C], cdt, tag="wt")
            b0_t = pers.tile([CH, 4], f32, tag="b0")
            b1_t = pers.tile([CH, 4], f32, tag="b1")
            bt_t = pers.tile([C, 1], f32, tag="bt")
            xbuf = pers.tile([C, BUFC], cdt, tag="xb")
            h0p = [pers.tile([CH, BUFC], cdt, tag=f"h0p{i}", name=f"h0p{i}")
                   for i in range(2)]
            h1p = [pers.tile([CH, BUFC], cdt, tag=f"h1p{i}", name=f"h1p{i}")
                   for i in range(2)]
            c0_t = pers.tile([CH, HW], f32, tag="c0")
            c1_t = pers.tile([CH, HW], f32, tag="c1")

            dma = (nc.gpsimd if MM_DT == "bf16" else nc.sync)

            for t_, d_ in ((w0_t, w0_d), (u0_t, u0_d), (w1_t, w1_d),
                           (u1_t, u1_d), (wt_t, wt_d)):
                dma.dma_start(t_[:], d_.ap())
            for t_, d_ in ((b0_t, b0_d), (b1_t, b1_d), (bt_t, bt_d)):
                nc.sync.dma_start(t_[:], d_.ap())

            def init_states():
                for buf in (xbuf, h0p[0], h0p[1], h1p[0], h1p[1]):
                    if MM_DT == "f32r":
                        np_ = buf.shape[0]
                        nc.sync.dma_start(buf[:], zz_d.ap()[:np_])
                    else:
                        nc.vector.memset(buf[:], 0.0)
                dma.dma_start(interior(h0p[0], BASE, H), h0_d.ap())
                dma.dma_start(interior(h1p[0], BASE, H), h1_d.ap())
                nc.sync.dma_start(c0_t[:], c0_d.ap())
                nc.sync.dma_start(c1_t[:], c1_d.ap())

            def mm_cast(ap):
                return ap

            def conv_gates(xin, kx, wi_t, hin, wh_t, b_t, c_t, hout):
                """One ConvLSTM cell: gates = conv(xin)+conv(hin)+b; update
                c_t in place; write new h into hout's interior.

                Chunks are processed in pairs sharing each tap's stationary
                weights (two back-to-back matmuls per weight load)."""
                for bi in range(0, len(CHUNKS), 2):
                    pair = CHUNKS[bi:bi + 2]
                    gtiles = [[None] * 4 for _ in pair]
                    for g in range(4):
                        pss = [psp.tile([CH, (r1 - r0) * WP], f32, tag="ps",
                                        name="ps") for (r0, r1) in pair]
                        taps = []
                        for ti in range(9):
                            o = ti * 4 * CH + g * CH
                            taps.append((wi_t[:kx, o:o + CH], xin, kx))
                        for ti in range(9):
                            o = ti * 4 * CH + g * CH
                            taps.append((wh_t[:, o:o + CH], hin, CH))
                        if xin is not xbuf:
                            # layer 1: h2h operand ready earlier; put it first
                            taps = taps[9:] + taps[:9]
                        for k, (lhs, src, kk) in enumerate(taps):
                            dy, dx = TAPS[k % 9]
                            for j, (r0, r1) in enumerate(pair):
                                s = BASE + r0 * WP + dy * WP + dx
                                cw = (r1 - r0) * WP
                                nc.tensor.matmul(pss[j][:], lhs,
                                                 src[:kk, s:s + cw],
                                                 start=(k == 0), stop=(k == 17))
                        for j, (r0, r1) in enumerate(pair):
                            nr = r1 - r0
                            gt = gtp.tile([CH, nr * W], f32, tag=f"g{g}",
                                          name=f"g{g}")
                            func = AF.Tanh if g == 2 else AF.Sigmoid
                            nc.scalar.activation(
                                gt[:].rearrange("p (r c) -> p r c", c=W),
                                pss[j][:].rearrange(
                                    "p (r c) -> p r c", c=WP)[:, :, 1:1 + W],
                                func, bias=b_t[:, g:g + 1])
                            gtiles[j][g] = gt
                    for j, (r0, r1) in enumerate(pair):
                        nr = r1 - r0
                        gi, gf, gg, go = gtiles[j]
                        csl = c_t[:, r0 * W:r1 * W]
                        nc.vector.tensor_mul(gg[:], gi[:], gg[:])   # i*g
                        nc.vector.tensor_mul(csl, gf[:], csl)       # f*c
                        nc.vector.tensor_add(csl, csl, gg[:])       # c
                        nc.scalar.activation(gf[:], csl, AF.Tanh)
                        nc.vector.tensor_mul(
                            interior(hout, BASE + r0 * WP, nr),
                            go[:].rearrange("p (r c) -> p r c", c=W),
                            gf[:].rearrange("p (r c) -> p r c", c=W))

            def conv_top(hin, tout):
                for bi in range(0, len(CHUNKS), 2):
                    pair = CHUNKS[bi:bi + 2]
                    pss = [psp.tile([C, (r1 - r0) * WP], f32, tag="ps",
                                    name="ps") for (r0, r1) in pair]
                    for ti in range(9):
                        dy, dx = TAPS[ti]
                        lhs = wt_t[:, ti * C:(ti + 1) * C]
                        for j, (r0, r1) in enumerate(pair):
                            s = BASE + r0 * WP + dy * WP + dx
                            cw = (r1 - r0) * WP
                            nc.tensor.matmul(pss[j][:], lhs, hin[:, s:s + cw],
                                             start=(ti == 0), stop=(ti == 8))
                    for j, (r0, r1) in enumerate(pair):
                        nr = r1 - r0
                        ot = osbp.tile([C, nr * W], f32, tag="ot", name="ot")
                        nc.scalar.activation(
                            ot[:].rearrange("p (r c) -> p r c", c=W),
                            pss[j][:].rearrange(
                                "p (r c) -> p r c", c=WP)[:, :, 1:1 + W],
                            AF.Identity, bias=bt_t[:, 0:1])
                        nc.sync.dma_start(tout[:, r0 * W:r1 * W], ot[:])

            def body():
                init_states()
                conv_top(h1p[0], out_d.ap()[0])
                for t in range(NSTEP):
                    dma.dma_start(interior(xbuf, BASE, H), xs_d.ap()[t])
                    conv_gates(xbuf, C, w0_t, h0p[t % 2], u0_t, b0_t, c0_t,
                               h0p[(t + 1) % 2])
                    conv_gates(h0p[(t + 1) % 2], CH, w1_t, h1p[t % 2], u1_t,
                               b1_t, c1_t, h1p[(t + 1) % 2])
                    conv_top(h1p[(t + 1) % 2], out_d.ap()[t + 1])

            if LOOP_N > 0:
                with tc.For_i(0, LOOP_N, 1):
                    body()
            else:
                body()

    nc.compile()
    return nc


def _get_nc():
    if "nc" not in _CACHE:
        _CACHE["nc"] = _build()
    return _CACHE["nc"]


def kernel(target, h0, c0, h1, c1,
           wi0, bi0, wh0, bh0,
           wi1, bi1, wh1, bh1,
           wtop, btop):
    from concourse.bass_utils import run_bass_kernel_spmd

    nc = _get_nc()

    target = np.asarray(target, np.float32)
    shared = {
        "w0": _prep_w(np.asarray(wi0, np.float32)),
        "u0": _prep_w(np.asarray(wh0, np.float32)),
        "w1": _prep_w(np.asarray(wi1, np.float32)),
        "u1": _prep_w(np.asarray(wh1, np.float32)),
        "wt": _prep_w(np.asarray(wtop, np.float32)),
        "b0": np.ascontiguousarray(
            (np.asarray(bi0) + np.asarray(bh0)).astype(np.float32)
            .reshape(4, CH).T),
        "b1": np.ascontiguousarray(
            (np.asarray(bi1) + np.asarray(bh1)).astype(np.float32)
            .reshape(4, CH).T),
        "bt": np.asarray(btop, np.float32).reshape(C, 1),
        "zz": np.zeros((CH, BUFC), np.float32),
    }
    in_maps = []
    for b in range(B):
        m = dict(shared)
        m["xs"] = np.ascontiguousarray(
            target[b, :NSTEP].reshape(NSTEP, C, HW))
        m["h0i"] = np.ascontiguousarray(np.asarray(h0, np.float32)[b].reshape(CH, HW))
        m["c0i"] = np.ascontiguousarray(np.asarray(c0, np.float32)[b].reshape(CH, HW))
        m["h1i"] = np.ascontiguousarray(np.asarray(h1, np.float32)[b].reshape(CH, HW))
        m["c1i"] = np.ascontiguousarray(np.asarray(c1, np.float32)[b].reshape(CH, HW))
        in_maps.append(m)

    res = run_bass_kernel_spmd(nc, in_maps, core_ids=list(range(B)))
    out = np.stack([res.results[b]["out"].reshape(T, C, H, W)
                    for b in range(B)])
    return out


# revision 21
# speedup vs baseline: 1.0887x; 1.0110x over previous
"""ConvLSTM decoder (2 ConvLSTM layers + top conv) on 8 Trainium2 cores.

Sharding: data-parallel over batch — B=8, one batch element per core,
weights replicated. The T=10 recurrence runs fully on-core.

Layout: images are stored in SBUF as a zero-padded flat row-major strip:
each 64-pixel row padded to 66 cols (1 zero col each side), 64 rows
contiguous, plus 68-col zero margins at both ends. A 3x3 'SAME' conv then
becomes 9 shifted matmuls accumulated in PSUM: for tap (dy,dx) the rhs is
the flat strip shifted by dy*66+dx.
"""

import numpy as np

B, T, C, H, W = 8, 10, 64, 64, 64
CH = 128
NSTEP = T - 1          # 9 recurrent steps
WP = W + 2             # padded row width
FLAT = H * WP          # 4224
MARG = 68              # >= 67 = max |tap offset|
BUFC = MARG + FLAT + MARG
BASE = MARG
HW = H * W             # 4096

# row chunks (r0, r1): 8x7 rows + 2x4 rows; max matmul N = 7*66 = 462 <= 512
CHUNKS = [(i * 7, i * 7 + 7) for i in range(8)] + [(56, 60), (60, 64)]

TAPS = [(dy, dx) for dy in (-1, 0, 1) for dx in (-1, 0, 1)]

MM_DT = "f32r"         # "f32" | "f32r" | "bf16"
LOOP_N = 0             # >0: wrap body in a hardware repeat loop (timing only)

_CACHE = {}


def _np_dt(mybir):
    if MM_DT == "bf16":
        return mybir.dt.bfloat16
    if MM_DT == "f32r":
        return mybir.dt.float32r
    return mybir.dt.float32


def _prep_w(w):
    # [O, I, 3, 3] -> [I, 9*O]; slice for (tap ti, 128-chunk g): ti*O + g*128
    O, I = w.shape[0], w.shape[1]
    return np.ascontiguousarray(
        w.transpose(1, 2, 3, 0).reshape(I, 9 * O).astype(np.float32))


def _build():
    import concourse.bass as bass
    import concourse.tile as tile
    from concourse import bacc, mybir

    f32 = mybir.dt.float32
    cdt = _np_dt(mybir)          # matmul-input dtype in SBUF
    AF = mybir.ActivationFunctionType

    nc = bacc.Bacc("TRN2", target_bir_lowering=False, debug=False,
                   num_devices=8)

    ddt = cdt if MM_DT == "f32r" else f32   # DRAM dtype for matmul operands
    xs_d = nc.dram_tensor("xs", [NSTEP, C, HW], ddt, kind="ExternalInput")
    h0_d = nc.dram_tensor("h0i", [CH, HW], ddt, kind="ExternalInput")
    c0_d = nc.dram_tensor("c0i", [CH, HW], f32, kind="ExternalInput")
    h1_d = nc.dram_tensor("h1i", [CH, HW], ddt, kind="ExternalInput")
    c1_d = nc.dram_tensor("c1i", [CH, HW], f32, kind="ExternalInput")
    w0_d = nc.dram_tensor("w0", [C, 9 * 4 * CH], ddt, kind="ExternalInput")
    u0_d = nc.dram_tensor("u0", [CH, 9 * 4 * CH], ddt, kind="ExternalInput")
    w1_d = nc.dram_tensor("w1", [CH, 9 * 4 * CH], ddt, kind="ExternalInput")
    u1_d = nc.dram_tensor("u1", [CH, 9 * 4 * CH], ddt, kind="ExternalInput")
    wt_d = nc.dram_tensor("wt", [CH, 9 * C], ddt, kind="ExternalInput")
    zz_d = nc.dram_tensor("zz", [CH, BUFC], ddt, kind="ExternalInput")
    b0_d = nc.dram_tensor("b0", [CH, 4], f32, kind="ExternalInput")
    b1_d = nc.dram_tensor("b1", [CH, 4], f32, kind="ExternalInput")
    bt_d = nc.dram_tensor("bt", [C, 1], f32, kind="ExternalInput")
    out_d = nc.dram_tensor("out", [T, C, HW], f32, kind="ExternalOutput")

    def interior(ap_2d, s0, nrow):
        # rows of 64 interior cols at stride 66 starting at flat offset s0
        return ap_2d[:, s0:s0 + nrow * WP].rearrange(
            "p (r c) -> p r c", c=WP)[:, :, 1:1 + W]

    with tile.TileContext(nc) as tc:
        with (
            tc.tile_pool(name="pers", bufs=1) as pers,
            tc.tile_pool(name="ps", bufs=8, space="PSUM") as psp,
            tc.tile_pool(name="gt", bufs=2) as gtp,
            tc.tile_pool(name="osb", bufs=1) as osbp,
        ):
            # --- persistent SBUF residents ---
            w0_t = pers.tile([C, 9 * 4 * CH], cdt, tag="w0")
            u0_t = pers.tile([CH, 9 * 4 * CH], cdt, tag="u0")
            w1_t = pers.tile([CH, 9 * 4 * CH], cdt, tag="w1")
            u1_t = pers.tile([CH, 9 * 4 * CH], cdt, tag="u1")
            wt_t = pers.tile([CH, 9 * C], cdt, tag="wt")
            b0_t = pers.tile([CH, 4], f32, tag="b0")
            b1_t = pers.tile([CH, 4], f32, tag="b1")
            bt_t = pers.tile([C, 1], f32, tag="bt")
            xbuf = pers.tile([C, BUFC], cdt, tag="xb")
            h0p = [pers.tile([CH, BUFC], cdt, tag=f"h0p{i}", name=f"h0p{i}")
                   for i in range(2)]
            h1p = [pers.tile([CH, BUFC], cdt, tag=f"h1p{i}", name=f"h1p{i}")
                   for i in range(2)]
            c0_t = pers.tile([CH, HW], f32, tag="c0")
            c1_t = pers.tile([CH, HW], f32, tag="c1")

            dma = (nc.gpsimd if MM_DT == "bf16" else nc.sync)

            for t_, d_ in ((w0_t, w0_d), (u0_t, u0_d), (w1_t, w1_d),
                           (u1_t, u1_d), (wt_t, wt_d)):
                dma.dma_start(t_[:], d_.ap())
            for t_, d_ in ((b0_t, b0_d), (b1_t, b1_d), (bt_t, bt_d)):
                nc.sync.dma_start(t_[:], d_.ap())

            def init_states():
                for buf in (xbuf, h0p[0], h0p[1], h1p[0], h1p[1]):
                    if MM_DT == "f32r":
                        np_ = buf.shape[0]
                        nc.sync.dma_start(buf[:], zz_d.ap()[:np_])
                    else:
                        nc.vector.memset(buf[:], 0.0)
                dma.dma_start(interior(h0p[0], BASE, H), h0_d.ap())
                dma.dma_start(interior(h1p[0], BASE, H), h1_d.ap())
                nc.sync.dma_start(c0_t[:], c0_d.ap())
                nc.sync.dma_start(c1_t[:], c1_d.ap())

            def mm_cast(ap):
                return ap

            def conv_gates(xin, kx, wi_t, hin, wh_t, b_t, c_t, hout):
                """One ConvLSTM cell: gates = conv(xin)+conv(hin)+b; update
                c_t in place; write new h into hout's interior.

                Chunks are processed in pairs sharing each tap's stationary
                weights (two back-to-back matmuls per weight load)."""
                for bi in range(0, len(CHUNKS), 2):
                    pair = CHUNKS[bi:bi + 2]
                    gtiles = [[None] * 4 for _ in pair]
                    for g in range(4):
                        pss = [psp.tile([CH, (r1 - r0) * WP], f32, tag="ps",
                                        name="ps") for (r0, r1) in pair]
                        taps = []
                        for ti in range(9):
                            o = ti * 4 * CH + g * CH
                            taps.append((wi_t[:kx, o:o + CH], xin, kx))
                        for ti in range(9):
                            o = ti * 4 * CH + g * CH
                            taps.append((wh_t[:, o:o + CH], hin, CH))
                        if xin is not xbuf:
                            # layer 1: h2h operand ready earlier; put it first
                            taps = taps[9:] + taps[:9]
                        for k, (lhs, src, kk) in enumerate(taps):
                            dy, dx = TAPS[k % 9]
                            for j, (r0, r1) in enumerate(pair):
                                s = BASE + r0 * WP + dy * WP + dx
                                cw = (r1 - r0) * WP
                                nc.tensor.matmul(pss[j][:], lhs,
                                                 src[:kk, s:s + cw],
                                                 start=(k == 0), stop=(k == 17))
                        for j, (r0, r1) in enumerate(pair):
                            nr = r1 - r0
                            gt = gtp.tile([CH, nr * W], f32, tag=f"g{g}",
                                          name=f"g{g}")
                            func = AF.Tanh if g == 2 else AF.Sigmoid
                            nc.scalar.activation(
                                gt[:].rearrange("p (r c) -> p r c", c=W),
                                pss[j][:].rearrange(
                                    "p (r c) -> p r c", c=WP)[:, :, 1:1 + W],
                                func, bias=b_t[:, g:g + 1])
                            gtiles[j][g] = gt
                    for j, (r0, r1) in enumerate(pair):
                        nr = r1 - r0
                        gi, gf, gg, go = gtiles[j]
                        csl = c_t[:, r0 * W:r1 * W]
                        nc.vector.tensor_mul(gg[:], gi[:], gg[:])   # i*g
                        nc.vector.tensor_mul(csl, gf[:], csl)       # f*c
                        nc.vector.tensor_add(csl, csl, gg[:])       # c
                        nc.scalar.activation(gf[:], csl, AF.Tanh)
                        nc.vector.tensor_mul(
                            interior(hout, BASE + r0 * WP, nr),
                            go[:].rearrange("p (r c) -> p r c", c=W),
                            gf[:].rearrange("p (r c) -> p r c", c=W))

            def conv_top(hin, tout):
                for bi in range(0, len(CHUNKS), 2):
                    pair = CHUNKS[bi:bi + 2]
                    pss = [psp.tile([C, (r1 - r0) * WP], f32, tag="ps",
                                    name="ps") for (r0, r1) in pair]
                    for ti in range(9):
                        dy, dx = TAPS[ti]
                        lhs = wt_t[:, ti * C:(ti + 1) * C]
                        for j, (r0, r1) in enumerate(pair):
                            s = BASE + r0 * WP + dy * WP + dx
                            cw = (r1 - r0) * WP
                            nc.tensor.matmul(pss[j][:], lhs, hin[:, s:s + cw],
                                             start=(ti == 0), stop=(ti == 8))
                    for j, (r0, r1) in enumerate(pair):
                        nr = r1 - r0
                        ot = osbp.tile([C, nr * W], f32, tag="ot", name="ot")
                        nc.scalar.activation(
                            ot[:].rearrange("p (r c) -> p r c", c=W),
                            pss[j][:].rearrange(
                                "p (r c) -> p r c", c=WP)[:, :, 1:1 + W],
                            AF.Identity, bias=bt_t[:, 0:1])
                        nc.sync.dma_start(tout[:, r0 * W:r1 * W], ot[:])

            def body():
                init_states()
                conv_top(h1p[0], out_d.ap()[0])
                for t in range(NSTEP):
                    dma.dma_start(interior(xbuf, BASE, H), xs_d.ap()[t])
                    conv_gates(xbuf, C, w0_t, h0p[t % 2], u0_t, b0_t, c0_t,
                               h0p[(t + 1) % 2])
                    conv_gates(h0p[(t + 1) % 2], CH, w1_t, h1p[t % 2], u1_t,
                               b1_t, c1_t, h1p[(t + 1) % 2])
                    conv_top(h1p[(t + 1) % 2], out_d.ap()[t + 1])

            if LOOP_N > 0:
                with tc.For_i(0, LOOP_N, 1):
                    body()
            else:
                body()

    nc.compile()
    return nc


def _get_nc():
    if "nc" not in _CACHE:
        _CACHE["nc"] = _build()
    return _CACHE["nc"]


def kernel(target, h0, c0, h1, c1,
           wi0, bi0, wh0, bh0,
           wi1, bi1, wh1, bh1,
           wtop, btop):
    from concourse.bass_utils import run_bass_kernel_spmd

    nc = _get_nc()

    target = np.asarray(target, np.float32)
    shared = {
        "w0": _prep_w(np.asarray(wi0, np.float32)),
        "u0": _prep_w(np.asarray(wh0, np.float32)),
        "w1": _prep_w(np.asarray(wi1, np.float32)),
        "u1": _prep_w(np.asarray(wh1, np.float32)),
        "wt": _prep_w(np.asarray(wtop, np.float32)),
        "b0": np.ascontiguousarray(
            (np.asarray(bi0) + np.asarray(bh0)).astype(np.float32)
            .reshape(4, CH).T),
        "b1": np.ascontiguousarray(
            (np.asarray(bi1) + np.asarray(bh1)).astype(np.float32)
            .reshape(4, CH).T),
        "bt": np.asarray(btop, np.float32).reshape(C, 1),
        "zz": np.zeros((CH, BUFC), np.float32),
    }
    in_maps = []
    for b in range(B):
        m = dict(shared)
        m["xs"] = np.ascontiguousarray(
            target[b, :NSTEP].reshape(NSTEP, C, HW))
        m["h0i"] = np.ascontiguousarray(np.asarray(h0, np.float32)[b].reshape(CH, HW))
        m["c0i"] = np.ascontiguousarray(np.asarray(c0, np.float32)[b].reshape(CH, HW))
        m["h1i"] = np.ascontiguousarray(np.asarray(h1, np.float32)[b].reshape(CH, HW))
        m["c1i"] = np.ascontiguousarray(np.asarray(c1, np.float32)[b].reshape(CH, HW))
        in_maps.append(m)

    res = run_bass_kernel_spmd(nc, in_maps, core_ids=list(range(B)))
    out = np.stack([res.results[b]["out"].reshape(T, C, H, W)
                    for b in range(B)])
    return out
